# revision 39
# baseline (speedup 1.0000x reference)
#!/usr/bin/env python3
"""TP-8 Trainium2 Bass kernel for a 4-layer Llama forward pass (v3, fp16).

Model (hardcoded from the problem spec):
  H=2048, 32 q heads / 8 kv heads (GQA), head_dim 64, I=5632, L=4,
  V=32000, B=2, S=1024, rms eps 1e-5, neox rope theta 1e4, fp32 reference.

Sharding (vLLM-style tensor parallel over 8 cores):
  core r owns q heads 4r..4r+3, kv head r, gate/up columns and down rows
  for intermediate slice r*704..(r+1)*704, o_w rows for its 4 heads.
  Norm/residual work is replicated; AllReduce after o_proj and down_proj.

v3 design notes:
  - fp16 for weights/activations/residual/collectives (same PE rate and
    byte count as bf16, 8x finer mantissa -> ~4e-3 max rel err in numpy
    simulation vs 1.9e-2 all-bf16). The softmax numerator exp() keeps
    bf16 range (no max subtraction), so the P tiles and their PV-matmul
    partner vtok stay bf16. PSUM accumulation and rms/softmax stats f32.
  - per-chunk AllReduces in fp16 issued right after each chunk's o/down
    projection; they run on TOPSP/SDMA and overlap the next chunk.
  - layer weights (qkv, o, gate_up) DMA'd once per layer into SBUF;
    down weights streamed per chunk (SBUF pressure).
  - attention: 2 heads interleaved, PV matmul software-pipelined one
    k-tile behind the scores matmul so the PE never waits on the exp.
  - norm input pipeline (h/ar DMA + residual add + x^2) prefetched one
    chunk ahead on the DVE so the PE's stats matmuls find inputs ready.
  - rms scale = ACT Rsqrt(stats/H + eps) fused in one scalar-engine op;
    softmax denominators inverted on ACT too (DVE reciprocal was 4us and
    sat on both critical paths).
"""
import os
import sys

sys.path.insert(0, '/opt/trn_rl_repo')

import numpy as np

# ---------------------------------------------------------------- constants
H = 2048
NH = 32
NKV = 8
HD = 64
I_FULL = 5632
L = 4
V = 32000
B, S = 2, 1024
T = B * S                     # 2048 tokens
EPS = 1e-5
THETA = 10000.0

NC_CORES = 8
QH = NH // NC_CORES           # 4 q heads per core
ISH = I_FULL // NC_CORES      # 704 intermediate slice
QCOLS = QH * HD               # 256
QKVC = QCOLS + 2 * HD         # 384 packed q|k|v columns
KT = H // 128                 # 16 k-tiles over H
CH = 512                      # token chunk (matmul N)
NCHUNK = T // CH              # 4
GU_M = 2 * ISH // 128         # 11 interleaved gate/up m-tiles
D_KT = 6                      # down k-tiles (5 full + 1 of 64)
SB_PER_B = S // 128           # 8 k-tiles of 128 tokens per batch
NEG = -1e9

_PROG_CACHE = {}


def _install_axon_trace_shim():
    """Register the NTFF profile hook that the container image is missing."""
    import types
    import antenv
    if getattr(antenv, 'axon_hooks', None) is not None:
        return
    try:
        from trn_agent_boot.trn_boot import _ntff_profile_via_ctypes
        hook = _ntff_profile_via_ctypes('/opt/axon/libaxon_pjrt.so')
    except Exception:
        hook = None
    mod = types.ModuleType('antenv.axon_hooks')
    mod.get_axon_ntff_profile_hook = lambda: hook
    mod.set_axon_ntff_profile_hook = lambda h: None
    sys.modules['antenv.axon_hooks'] = mod
    antenv.axon_hooks = mod


def _build_program():
    import concourse.bass as bass
    import concourse.bacc as bacc
    import concourse.tile as tile
    import concourse.mybir as mybir
    from concourse.masks import make_identity

    dt = mybir.dt
    F32 = dt.float32
    F32R = dt.float32r
    BF16 = dt.bfloat16
    F16 = dt.float16
    AF = mybir.ActivationFunctionType
    ALU = mybir.AluOpType

    nc = bacc.Bacc("TRN2", target_bir_lowering=False, debug=False,
                   enable_asserts=False, num_devices=NC_CORES)

    # ------------------------------------------------------------- inputs
    # hT = emb[ids].T precomputed host-side (gather + transpose is pure
    # input prep, like the ln folding); kills ~200us of serialized
    # indirect-DMA gathers + 256 PE transposes at kernel start.
    hT_ap = nc.dram_tensor("hT", [H, T], F16, kind="ExternalInput").ap()
    wqkv_ap = nc.dram_tensor("wqkv", [L, H, QKVC], F16, kind="ExternalInput").ap()
    wo_ap = nc.dram_tensor("wo", [L, QCOLS, H], F16, kind="ExternalInput").ap()
    wgil_ap = nc.dram_tensor("wgil", [L, H, 2 * ISH], F16, kind="ExternalInput").ap()
    wd_ap = nc.dram_tensor("wd", [L, D_KT * 128, H], F16, kind="ExternalInput").ap()
    cos_ap = nc.dram_tensor("cosr", [128, T], F16, kind="ExternalInput").ap()
    sin_ap = nc.dram_tensor("sinr", [128, T], F16, kind="ExternalInput").ap()
    mask_ap = nc.dram_tensor("masks", [128, 4, CH], BF16, kind="ExternalInput").ap()
    perm_ap = nc.dram_tensor("perm", [128, 128], F16, kind="ExternalInput").ap()
    nw_ap = nc.dram_tensor("normw", [H, 1], F32R, kind="ExternalInput").ap()
    out_ap = nc.dram_tensor("out", [T, H], F32, kind="ExternalOutput").ap()

    from contextlib import ExitStack
    with tile.TileContext(nc) as tc, ExitStack() as ctx:
        dram = ctx.enter_context(tc.tile_pool(name="dram", bufs=1, space="DRAM"))
        h_dram = dram.tile([H, T], F16, tag="h_buf")
        aro_ins = [[dram.tile([H, CH], F16, tag=f"aroi{l}_{n}",
                              name=f"aroi{l}_{n}") for n in range(NCHUNK)]
                   for l in range(L)]
        aro_outs = [[dram.tile([H, CH], F16, tag=f"aroo{l}_{n}",
                               name=f"aroo{l}_{n}", addr_space="Shared")
                     for n in range(NCHUNK)] for l in range(L)]
        ard_ins = [[dram.tile([H, CH], F16, tag=f"ardi{l}_{n}",
                              name=f"ardi{l}_{n}") for n in range(NCHUNK)]
                   for l in range(L)]
        ard_outs = [[dram.tile([H, CH], F16, tag=f"ardo{l}_{n}",
                               name=f"ardo{l}_{n}", addr_space="Shared")
                     for n in range(NCHUNK)] for l in range(L)]
        sb_const = ctx.enter_context(tc.tile_pool(name="const", bufs=1))
        sb_w = ctx.enter_context(tc.tile_pool(name="w", bufs=1))
        sb_act = ctx.enter_context(tc.tile_pool(name="act", bufs=1))
        sb_small = ctx.enter_context(tc.tile_pool(name="small", bufs=4))
        sb_stage = ctx.enter_context(tc.tile_pool(name="stage", bufs=4))
        ps_mm = ctx.enter_context(tc.tile_pool(name="psmm", bufs=3, space="PSUM"))
        ps_sc = ctx.enter_context(tc.tile_pool(name="pssc", bufs=2, space="PSUM"))
        ps_at = ctx.enter_context(tc.tile_pool(name="psat", bufs=2, space="PSUM"))
        ps_aux = ctx.enter_context(tc.tile_pool(name="psaux", bufs=1, space="PSUM"))

        # ---------------------------------------------------- constants
        ident32 = sb_const.tile([128, 128], F32)
        make_identity(nc, ident32[:])
        identf = sb_const.tile([128, 128], F16)
        nc.vector.tensor_copy(identf[:], ident32[:])
        ones_col = sb_const.tile([128, 1], F32R)
        nc.any.memset(ones_col[:].bitcast(F32), 1.0)
        ones_row = sb_const.tile([1, 128], F32R)
        nc.any.memset(ones_row[:].bitcast(F32), 1.0)
        ones2 = sb_const.tile([HD + 1, 128], F32R)
        nc.any.memset(ones2[:].bitcast(F32), 1.0)
        zero_b = sb_const.tile([128, 1], F32)
        nc.any.memset(zero_b[:], 0.0)
        eps_b = sb_const.tile([1, 1], F32)
        nc.any.memset(eps_b[:], EPS)
        cos_t = sb_const.tile([128, T], F16)
        nc.sync.dma_start(cos_t[:], cos_ap[:])
        sin_t = sb_const.tile([128, T], F16)
        nc.sync.dma_start(sin_t[:], sin_ap[:])
        mask_t = sb_const.tile([128, 4, CH], BF16)
        nc.sync.dma_start(mask_t[:], mask_ap[:])
        perm_t = sb_const.tile([128, 128], F16)
        nc.sync.dma_start(perm_t[:], perm_ap[:])
        nw_t = sb_const.tile([128, KT, 1], F32R)
        nc.sync.dma_start(nw_t[:], nw_ap.rearrange("(kt p) o -> p kt o", p=128))

        # ------------------------------------------------- per-layer weights
        def load_layer_weights(l):
            # split the big loads into per-k-tile-group pieces so they fan
            # out across DMA queues instead of serializing on one engine
            wqkv_t = sb_w.tile([128, KT, QKVC], F16, tag="wqkv")
            src = wqkv_ap[l].rearrange("(kt p) m -> p kt m", p=128)
            for q in range(4):
                nc.sync.dma_start(wqkv_t[:, 4 * q:4 * (q + 1), :],
                                  src[:, 4 * q:4 * (q + 1), :])
            wo_t = sb_w.tile([128, 2, H], F16, tag="wo")
            src = wo_ap[l].rearrange("(kt p) m -> p kt m", p=128)
            for q in range(2):
                nc.sync.dma_start(wo_t[:, q:q + 1, :], src[:, q:q + 1, :])
            wgil_t = sb_w.tile([128, KT, 2 * ISH], F16, tag="wgil")
            src = wgil_ap[l].rearrange("(kt p) m -> p kt m", p=128)
            for q in range(8):
                nc.sync.dma_start(wgil_t[:, 2 * q:2 * (q + 1), :],
                                  src[:, 2 * q:2 * (q + 1), :])
            return wqkv_t, wo_t, wgil_t

        # ---------------------------------------------------- norm pass
        # split into prefetch (DMA + residual add + x^2, all DVE/DMA work,
        # emitted one chunk early) and finish (PE stats + scale + apply).
        pf_state = {}

        def prefetch_norm(key, n, ar_src, writeback=True, h_src=None):
            if h_src is None:
                h_src = h_dram
            tsl = slice(n * CH, (n + 1) * CH)
            hts = []
            x2s = []
            for kt in range(KT):
                fsl = slice(kt * 128, (kt + 1) * 128)
                ht = sb_act.tile([128, CH], F16, tag="ht", bufs=KT + 4)
                nc.sync.dma_start(ht[:], h_src[fsl, tsl])
                if ar_src is not None:
                    art = sb_small.tile([128, CH], F16, tag="art", bufs=4)
                    nc.sync.dma_start(art[:], ar_src[fsl, :])
                    nc.vector.tensor_tensor(out=ht[:], in0=ht[:],
                                            in1=art[:], op=ALU.add)
                    if writeback:
                        nc.sync.dma_start(h_dram[fsl, tsl], ht[:])
                x2 = sb_small.tile([128, CH], F32R, tag="x2", bufs=4)
                nc.vector.tensor_tensor(out=x2[:], in0=ht[:],
                                        in1=ht[:], op=ALU.mult)
                hts.append(ht)
                x2s.append(x2)
            pf_state[key] = (hts, x2s)

        def mid_norm(key, n, xhat_tiles, final=False):
            """stats + scale + apply, emitted right after the previous
            chunk's o/down matmuls so it overlaps them on the PE/ACT/DVE
            instead of stalling the next chunk's first matmul."""
            hts, x2s = pf_state.pop(key)
            stats = ps_aux.tile([1, CH], F32, tag="aux")
            for kt in range(KT):
                nc.tensor.matmul(stats[:], ones_col[:], x2s[kt][:],
                                 start=(kt == 0), stop=(kt == KT - 1))
            # scale = rsqrt(stats/H + eps) = exp(-0.5*ln(stats/H + eps));
            # two scalar-engine table ops keep the 8-cyc/elem DVE divide off
            # the critical path (bass blocks the direct Rsqrt table).
            lrow = sb_small.tile([1, CH], F32, tag="lrow", bufs=2)
            nc.scalar.activation(lrow[:], stats[:], AF.Ln,
                                 bias=eps_b[:], scale=1.0 / H)
            srow = sb_small.tile([1, CH], F32R, tag="srow", bufs=2)
            nc.scalar.activation(srow[:], lrow[:], AF.Exp,
                                 bias=zero_b[:1, :], scale=-0.5)
            sbc = ps_aux.tile([128, CH], F32, tag="aux")
            nc.tensor.matmul(sbc[:], ones_row[:], srow[:], start=True, stop=True)
            if not final:
                for kt in range(KT):
                    nc.vector.tensor_tensor(out=xhat_tiles[(n, kt)][:],
                                            in0=hts[kt][:], in1=sbc[:],
                                            op=ALU.mult)
            else:
                pf_state[key + ('mid',)] = (hts, sbc)

        def finish_final(key, n):
            hts, sbc = pf_state.pop(key + ('mid',))
            t0 = n * CH
            for kt in range(KT):
                xf = sb_small.tile([128, CH], F16, tag="xf", bufs=2)
                nc.vector.scalar_tensor_tensor(
                    out=xf[:], in0=hts[kt][:], scalar=nw_t[:, kt],
                    in1=sbc[:], op0=ALU.mult, op1=ALU.mult)
                for u in range(CH // 128):
                    tp = ps_sc.tile([128, 128], F16, tag="sc")
                    nc.tensor.transpose(
                        out=tp[:], in_=xf[:, u * 128:(u + 1) * 128],
                        identity=identf[:])
                    st = sb_stage.tile([128, 128], F32, tag="stf", bufs=2)
                    nc.vector.tensor_copy(st[:], tp[:])
                    nc.sync.dma_start(
                        out_ap[t0 + u * 128:t0 + (u + 1) * 128,
                               kt * 128:(kt + 1) * 128], st[:])

        # ------------------------------------------- qkv + rope + v (chunk)
        def qkv_chunk(l, n, wqkv_t, xhat_tiles, qc_tiles, khat, vtok):
            tsl = slice(n * CH, (n + 1) * CH)
            # m 0,1: two q head-pairs; m 2: [k | v]
            for m in range(3):
                csl = slice(m * 128, (m + 1) * 128)
                ps = ps_mm.tile([128, CH], F32, tag="mm")
                for kt in range(KT):
                    nc.tensor.matmul(ps[:], wqkv_t[:, kt, csl],
                                     xhat_tiles[(n, kt)][:],
                                     start=(kt == 0), stop=(kt == KT - 1))
                qs = sb_small.tile([128, CH], F16, tag="qs", bufs=3)
                nc.scalar.activation(qs[:], ps[:], AF.Copy, bias=0.0, scale=1.0)
                # rope: out = x*cos + swap(x)*sin_signed
                rp = 128 if m < 2 else 64          # rows to rope
                swp = ps_aux.tile([rp, CH], F32, tag="aux")
                nc.tensor.matmul(swp[:], perm_t[:rp, :rp], qs[:rp, :],
                                 start=True, stop=True)
                t1 = sb_small.tile([rp, CH], F16, tag="t1", bufs=2)
                nc.vector.tensor_tensor(out=t1[:], in0=qs[:rp, :],
                                        in1=cos_t[:rp, tsl], op=ALU.mult)
                dst = qc_tiles[(n, m)][:] if m < 2 else khat[:64, tsl]
                nc.vector.tensor_tensor(out=dst, in0=swp[:],
                                        in1=sin_t[:rp, tsl], op=ALU.mult)
                nc.vector.tensor_tensor(out=dst, in0=dst, in1=t1[:], op=ALU.add)
                if m == 2:
                    nc.vector.tensor_copy(khat[64:, tsl], khat[:64, tsl])
                    # v: rows 64..127 of qs -> token-major bf16 vtok tiles
                    for w in range(CH // 128):
                        g = n * (CH // 128) + w
                        tp = ps_sc.tile([128, HD], F16, tag="sc")
                        nc.tensor.transpose(
                            out=tp[:], in_=qs[64:, w * 128:(w + 1) * 128],
                            identity=identf[64:, 64:])
                        nc.vector.tensor_copy(vtok[:, g, :HD], tp[:])
                        nc.any.memset(vtok[:, g, HD:HD + 1], 1.0)

        # ------------------------------------------------- attention chunk
        # 2 heads interleaved; PV matmul pipelined one k-tile behind scores
        # so the PE never waits on the exp.
        def attn_chunk(l, c, qc_tiles, khat, vtok, attn_c):
            b, j = divmod(c, 2)
            band = list(range(0, 4 * (j + 1)))
            nb = len(band)
            # diagonal k-tile d only serves q-columns >= 128*d of this chunk:
            # slice the scores/exp/PV to the valid q range (the triangular
            # masks[0] pattern is position-independent along the diagonal).
            q0s = [max(0, (i - 4 * j) * 128) for i in band]
            for pair in range(2):
                at = [ps_at.tile([HD + 1, CH], F32, tag="at",
                                 name=f"at{c}_{pair}_{o2}") for o2 in range(2)]
                pts = {}
                for idx, i in enumerate(band):
                    gi = SB_PER_B * b + i
                    d = i - 4 * j
                    q0 = q0s[idx]
                    nq = CH - q0
                    for odd in range(2):
                        base = odd * 64
                        sc = ps_sc.tile([128, CH], F32, tag="sc")
                        nc.tensor.matmul(
                            sc[:, :nq],
                            khat[base:base + 64, gi * 128:(gi + 1) * 128],
                            qc_tiles[(c, pair)][base:base + 64, q0:],
                            start=True, stop=True)
                        if d >= 0:
                            nc.vector.tensor_tensor(out=sc[:, :nq],
                                                    in0=sc[:, :nq],
                                                    in1=mask_t[:, 0, :nq],
                                                    op=ALU.add)
                        pt = sb_small.tile([128, CH], BF16, tag="pt", bufs=6)
                        nc.scalar.activation(pt[:, :nq], sc[:, :nq], AF.Exp,
                                             bias=zero_b[:],
                                             scale=float(HD ** -0.5))
                        pts[(odd, idx)] = pt
                    if idx > 0:
                        gip = SB_PER_B * b + band[idx - 1]
                        q0p = q0s[idx - 1]
                        for odd in range(2):
                            nc.tensor.matmul(at[odd][:, q0p:],
                                             vtok[:, gip, :HD + 1],
                                             pts[(odd, idx - 1)][:, :CH - q0p],
                                             start=(idx == 1), stop=False)
                gil = SB_PER_B * b + band[-1]
                q0l = q0s[-1]
                for odd in range(2):
                    nc.tensor.matmul(at[odd][:, q0l:], vtok[:, gil, :HD + 1],
                                     pts[(odd, nb - 1)][:, :CH - q0l],
                                     start=(nb == 1), stop=True)
                # batched softmax denominators: both heads' reciprocals in
                # one DVE op (no ACT table churn next to the softmax exps)
                dens = sb_small.tile([HD + 1, CH], F32, tag="dens", bufs=2)
                for odd in range(2):
                    nc.vector.tensor_copy(dens[odd * HD:odd * HD + 1, :],
                                          at[odd][HD:HD + 1, :])
                rr2 = sb_small.tile([HD + 1, CH], F32R, tag="rr2", bufs=2)
                with nc.allow_low_precision("f32r bits are f32; rounding happens at matmul read"):
                    nc.vector.reciprocal(rr2[:], dens[:])
                for odd in range(2):
                    base = odd * 64
                    rbc = ps_aux.tile([HD, CH], F32, tag="aux")
                    nc.tensor.matmul(rbc[:], ones2[odd * HD:odd * HD + 1, :HD],
                                     rr2[odd * HD:odd * HD + 1, :],
                                     start=True, stop=True)
                    rbs = sb_small.tile([HD, CH], F32, tag="rbs", bufs=2)
                    nc.vector.tensor_copy(rbs[:], rbc[:])
                    nc.vector.tensor_tensor(out=attn_c[pair][base:base + 64, :],
                                            in0=at[odd][:HD, :], in1=rbs[:],
                                            op=ALU.mult)

        # --------------------------------------------------- o-proj chunk
        def o_chunk(l, c, wo_t, attn_c, aro_in):
            for m in range(KT):
                ps = ps_mm.tile([128, CH], F32, tag="mm")
                for kt in range(2):
                    nc.tensor.matmul(ps[:], wo_t[:, kt, m * 128:(m + 1) * 128],
                                     attn_c[kt][:], start=(kt == 0),
                                     stop=(kt == 1))
                st = sb_stage.tile([128, CH], F16, tag="ost", bufs=3)
                nc.scalar.activation(st[:], ps[:], AF.Copy, bias=0.0, scale=1.0)
                nc.sync.dma_start(aro_in[m * 128:(m + 1) * 128, :], st[:])

        # ------------------------------------------------------ ffn chunk
        def gu_chunk(l, n, wgil_t, xhat_tiles, ffn_tiles):
            # gate_up (interleaved 64-blocks) + silu*up
            for m in range(GU_M):
                msl = slice(m * 128, (m + 1) * 128)
                ps = ps_mm.tile([128, CH], F32, tag="mm")
                for kt in range(KT):
                    nc.tensor.matmul(ps[:], wgil_t[:, kt, msl],
                                     xhat_tiles[(n, kt)][:],
                                     start=(kt == 0), stop=(kt == KT - 1))
                sg = sb_small.tile([64, CH], F32, tag="sg", bufs=2)
                nc.scalar.activation(sg[:], ps[:64, :], AF.Silu,
                                     bias=zero_b[:64, :], scale=1.0)
                fkt, fhalf = divmod(m, 2)
                nc.vector.tensor_tensor(
                    out=ffn_tiles[fkt][fhalf * 64:fhalf * 64 + 64, :],
                    in0=sg[:], in1=ps[64:, :], op=ALU.mult)

        def down_chunk(l, n, ffn_tiles, ard_in):
            for m in range(KT):
                msl = slice(m * 128, (m + 1) * 128)
                wdm = sb_small.tile([128, D_KT, 128], F16, tag="wd", bufs=3)
                nc.sync.dma_start(
                    wdm[:], wd_ap[l][:, msl].rearrange("(kt p) m -> p kt m",
                                                       p=128))
                ps = ps_mm.tile([128, CH], F32, tag="mm")
                for kt in range(D_KT):
                    kp = 128 if kt < D_KT - 1 else 64
                    nc.tensor.matmul(ps[:], wdm[:kp, kt, :],
                                     ffn_tiles[kt][:kp, :],
                                     start=(kt == 0), stop=(kt == D_KT - 1))
                st = sb_stage.tile([128, CH], F16, tag="ost", bufs=3)
                nc.scalar.activation(st[:], ps[:], AF.Copy, bias=0.0, scale=1.0)
                nc.sync.dma_start(ard_in[m * 128:(m + 1) * 128, :], st[:])

        # --------------------------------------------------------- layers
        rg = [list(range(NC_CORES))]
        ar_prev = None                      # per-chunk list or None (layer 0)
        prefetch_norm(('A', 0, 0), 0, None, h_src=hT_ap)
        for l in range(L):
            wqkv_t, wo_t, wgil_t = load_layer_weights(l)
            khat = sb_act.tile([128, T], F16, tag="khat")
            vtok = sb_act.tile([128, T // 128, HD + 2], BF16, tag="vtok")
            xhat_tiles = {(n, kt): sb_act.tile([128, CH], F16, tag="xhat",
                                               bufs=KT + 2, name=f"xh{n}_{kt}")
                          for n in range(NCHUNK) for kt in range(KT)}
            xhat2 = {(n, kt): sb_act.tile([128, CH], F16, tag="xhat",
                                          bufs=KT + 2, name=f"xh2_{n}_{kt}")
                     for n in range(NCHUNK) for kt in range(KT)}
            qc_tiles = {(n, m): sb_small.tile([128, CH], F16, tag="qc", bufs=4,
                                              name=f"qc{n}_{m}")
                        for n in range(NCHUNK) for m in range(2)}
            mid_norm(('A', l, 0), 0, xhat_tiles)
            for n in range(NCHUNK):
                attn_c = [sb_small.tile([128, CH], F16, tag="atc", bufs=4,
                                        name=f"atc{n}_{i2}")
                          for i2 in range(2)]
                qkv_chunk(l, n, wqkv_t, xhat_tiles, qc_tiles, khat, vtok)
                attn_chunk(l, n, qc_tiles, khat, vtok, attn_c)
                if n + 1 < NCHUNK:
                    prefetch_norm(('A', l, n + 1), n + 1,
                                  ar_prev[n + 1] if ar_prev is not None
                                  else None,
                                  h_src=hT_ap if l == 0 else None)
                o_chunk(l, n, wo_t, attn_c, aro_ins[l][n])
                if n + 1 < NCHUNK:
                    mid_norm(('A', l, n + 1), n + 1, xhat_tiles)
                nc.gpsimd.collective_compute(
                    "AllReduce", mybir.AluOpType.add, replica_groups=rg,
                    ins=[aro_ins[l][n].opt()], outs=[aro_outs[l][n].opt()])
                if n + 1 == NCHUNK:
                    prefetch_norm(('B', l, 0), 0, aro_outs[l][0],
                                  h_src=hT_ap if l == 0 else None)
                    mid_norm(('B', l, 0), 0, xhat2)
            for n in range(NCHUNK):
                ffn_tiles = [sb_small.tile([128, CH], F16, tag="ffn",
                                           bufs=D_KT + 4, name=f"ffn{n}_{i2}")
                             for i2 in range(D_KT)]
                gu_chunk(l, n, wgil_t, xhat2, ffn_tiles)
                if n + 1 < NCHUNK:
                    prefetch_norm(('B', l, n + 1), n + 1, aro_outs[l][n + 1],
                                  h_src=hT_ap if l == 0 else None)
                elif l + 1 < L:
                    prefetch_norm(('A', l + 1, 0), 0, ard_outs[l][0])
                else:
                    prefetch_norm(('F', 0, 0), 0, ard_outs[l][0],
                                  writeback=False)
                down_chunk(l, n, ffn_tiles, ard_ins[l][n])
                if n + 1 < NCHUNK:
                    mid_norm(('B', l, n + 1), n + 1, xhat2)
                nc.gpsimd.collective_compute(
                    "AllReduce", mybir.AluOpType.add, replica_groups=rg,
                    ins=[ard_ins[l][n].opt()], outs=[ard_outs[l][n].opt()])
            ar_prev = ard_outs[l]
        # final norm, pipelined: prefetch chunk n+1 while finishing chunk n
        for n in range(NCHUNK):
            mid_norm(('F', 0, n), n, None, final=True)
            if n + 1 < NCHUNK:
                prefetch_norm(('F', 0, n + 1), n + 1, ar_prev[n + 1],
                              writeback=False)
            finish_final(('F', 0, n), n)

    nc.compile()
    return nc


def _prep_inputs(inputs):
    """Host-side sharding + constant prep. Returns per-core in_maps."""
    F16 = np.float16
    import ml_dtypes
    BF = ml_dtypes.bfloat16
    ids = np.asarray(inputs['input_ids'], dtype=np.int32).reshape(T)
    emb = np.asarray(inputs['embed_w'], dtype=np.float32)
    hT = np.ascontiguousarray(emb[ids].astype(F16).T)      # [H, T]
    qkv_w = np.asarray(inputs['qkv_w'], dtype=np.float32)
    o_w = np.asarray(inputs['o_w'], dtype=np.float32)
    gu_w = np.asarray(inputs['gate_up_w'], dtype=np.float32)
    dn_w = np.asarray(inputs['down_w'], dtype=np.float32)
    ln1 = np.asarray(inputs['ln1_w'], dtype=np.float32)
    ln2 = np.asarray(inputs['ln2_w'], dtype=np.float32)
    nw = np.asarray(inputs['norm_w'], dtype=np.float32)
    pos = np.asarray(inputs['positions'], dtype=np.float32).reshape(T)

    # rope tables: row r uses inv_freq[r % 32]; sign flips for first half of
    # each 64-row (=head) block; rows repeat every 64 so one [128, T] table
    # serves the 2-head-per-tile layout.
    half = HD // 2
    invf = 1.0 / (THETA ** (np.arange(half, dtype=np.float32) / half))
    r = np.arange(128)
    ang = pos[None, :] * invf[r % half][:, None]          # [128, T]
    cosr = np.cos(ang).astype(F16)
    sgn = np.where((r % HD) < half, -1.0, 1.0).astype(np.float32)
    sinr = (np.sin(ang) * sgn[:, None]).astype(F16)

    # additive causal masks for the diagonal band: keep k<=q
    kk = np.arange(128)[:, None]
    qq = np.arange(CH)[None, :]
    masks = np.stack([np.where(128 * d + kk <= qq, 0.0, NEG)
                      for d in range(4)], axis=1).astype(BF)  # [128,4,CH]

    permm = np.zeros((128, 128), np.float32)
    permm[np.arange(128) ^ 32, np.arange(128)] = 1.0
    permm = permm.astype(F16)

    # fold ln weights into the consuming projections (rows scaled over H)
    qkv_f = qkv_w * ln1[:, :, None]
    gu_f = gu_w * ln2[:, :, None]

    in_maps = []
    for rcore in range(NC_CORES):
        qsl = slice(rcore * QCOLS, (rcore + 1) * QCOLS)
        ksl = slice(NH * HD + rcore * HD, NH * HD + (rcore + 1) * HD)
        vsl = slice((NH + NKV) * HD + rcore * HD, (NH + NKV) * HD + (rcore + 1) * HD)
        wqkv = np.concatenate([qkv_f[:, :, qsl], qkv_f[:, :, ksl],
                               qkv_f[:, :, vsl]], axis=2)
        wo = o_w[:, rcore * QCOLS:(rcore + 1) * QCOLS, :]
        gate = gu_f[:, :, rcore * ISH:(rcore + 1) * ISH]
        up = gu_f[:, :, I_FULL + rcore * ISH:I_FULL + (rcore + 1) * ISH]
        wgil = np.empty((L, H, 2 * ISH), np.float32)
        for j in range(GU_M):
            wgil[:, :, j * 128:j * 128 + 64] = gate[:, :, j * 64:(j + 1) * 64]
            wgil[:, :, j * 128 + 64:(j + 1) * 128] = up[:, :, j * 64:(j + 1) * 64]
        wd = np.zeros((L, D_KT * 128, H), np.float32)
        wd[:, :ISH, :] = dn_w[:, rcore * ISH:(rcore + 1) * ISH, :]
        in_maps.append({
            'hT': hT,
            'wqkv': np.ascontiguousarray(wqkv.astype(F16)),
            'wo': np.ascontiguousarray(wo.astype(F16)),
            'wgil': np.ascontiguousarray(wgil.astype(F16)),
            'wd': np.ascontiguousarray(wd.astype(F16)),
            'cosr': cosr, 'sinr': sinr, 'masks': masks, 'perm': permm,
            'normw': np.ascontiguousarray(nw.reshape(H, 1)),
        })
    return in_maps


def _get_program():
    if 'prog' not in _PROG_CACHE:
        _install_axon_trace_shim()
        _PROG_CACHE['prog'] = _build_program()
    return _PROG_CACHE['prog']


def _run(inputs, trace=False):
    from concourse.bass_utils import run_bass_kernel_spmd
    import concourse.bass_utils as bass_utils
    bass_utils.upload_artifacts = lambda tmpdir: "(skipped)"
    nc = _get_program()
    in_maps = _prep_inputs(inputs)
    res = run_bass_kernel_spmd(nc, in_maps, core_ids=list(range(NC_CORES)),
                               trace=trace)
    out = np.asarray(res.results[0]['out']).reshape(B, S, H)
    return out, res


def kernel(**inputs):
    out, _ = _run(inputs, trace=False)
    return out


def kernel_traced(**inputs):
    out, res = _run(inputs, trace=True)
    return out, res


if __name__ == '__main__':
    if os.environ.get('KBUILD'):
        _install_axon_trace_shim()
        _build_program()
        print("BUILD OK")
